# revision 32
# baseline (speedup 1.0000x reference)
"""Causal self-attention (B=2, T=2048, C=1024, 16 heads x 64) on 8 TRN2 cores.

Sharding: tensor-parallel over heads (2 heads/core). Each core computes its
heads' QKV projection, causal attention, and a partial output projection
(contraction over its 128 attn columns); the host sums the 8 partials.

v3 design notes (HAM-warmth + engine-balance rewrite of v2):
  - PE warm-up: 6 dummy matmuls at t~0 so the HAM clock gate reaches 8/8
    (2.4 GHz) right as the first real matmuls issue.
  - x DMA'd in 4 token-chunks per batch; QKV runs token-chunk-pipelined
    (c-inner accumulation, one PSUM bank per chunk) so attention for batch 0
    starts ~6us in. Remaining chunks + all of batch 1's QKV/V-transposes are
    emitted as PE filler units INSIDE the attention loop (tensor-queue FIFO =
    schedule), keeping the PE dense so HAM never re-throttles.
  - Attention software pipeline: scores(g) -> exp(g) [scalar] -> mask(g)
    [gpsimd] -> PV(g), with PV emitted one group behind scores so the PE
    never head-blocks on the exp stream (the scalar engine's ~91us of exp is
    the attention-phase clock; it must never starve).
  - PV keeps the ones-column trick (ones column last -> PSUM row 64; DVE/ACT
    partition bases must be quadrant-aligned, and reciprocal_approx_* needs
    base 0, hence the l2 bounce).
  - Engine balance: exp on scalar; masks on gpsimd; PSUM evacuations split
    scalar/vector so neither exceeds the tensor engine's ~106us of matmul.
  - Drains of qc N are emitted after qc N+1's first exps so the scalar queue
    never head-blocks waiting for PV.
"""

import os
from collections import deque

import numpy as np
import ml_dtypes

B = 2
T = 2048
C = 1024
N_HEADS = 16
D = 64
NCORES = 8
P = 128
BT = B * T
SCALE = D ** -0.5
NCHUNK = 4          # token chunks per batch for QKV pipeline
CH = T // NCHUNK    # 512

_bf16 = ml_dtypes.bfloat16

_COMPILED = None
LAST_RESULTS = None  # stashed BassKernelResults for test harness introspection


def _build():
    import concourse.bass as bass
    import concourse.mybir as mybir
    import concourse.tile as tile
    from concourse import bacc

    f32 = mybir.dt.float32
    bf16 = mybir.dt.bfloat16

    nc = bacc.Bacc("TRN2", target_bir_lowering=False, debug=False,
                   num_devices=NCORES)

    xT_d = nc.dram_tensor("xT", [P, B, NCHUNK, 8, CH], bf16,
                          kind="ExternalInput")
    wqkvT_d = nc.dram_tensor("wqkvT", [P, 8, 384], bf16, kind="ExternalInput")
    woutT_d = nc.dram_tensor("woutT", [P, C], bf16, kind="ExternalInput")
    btri_d = nc.dram_tensor("btri", [P, P], bf16, kind="ExternalInput")
    sel_d = nc.dram_tensor("sel", [65, P], bf16, kind="ExternalInput")
    ident_d = nc.dram_tensor("ident", [P, P], bf16, kind="ExternalInput")
    out_d = nc.dram_tensor("out", [BT, C], bf16, kind="ExternalOutput")

    Exp = mybir.ActivationFunctionType.Exp

    with tile.TileContext(nc) as tc:
        with (
            tc.tile_pool(name="const", bufs=1) as const_pool,
            tc.tile_pool(name="xb", bufs=2) as xb_pool,
            tc.tile_pool(name="qkv", bufs=2) as qkv_pool,
            tc.tile_pool(name="pt", bufs=4) as pt_pool,
            tc.tile_pool(name="attnT", bufs=2) as attnT_pool,
            tc.tile_pool(name="rl", bufs=2) as rl_pool,
            tc.tile_pool(name="osb", bufs=3) as osb_pool,
            tc.tile_pool(name="st", bufs=2, space="PSUM") as st_pool,
            tc.tile_pool(name="pv", bufs=1, space="PSUM") as pv_pool,
            tc.tile_pool(name="ps", bufs=2, space="PSUM") as ps_pool,
        ):
            # ---- constants ----
            wqkvT = const_pool.tile([P, 8, 384], bf16, tag="wqkvT")
            woutT = const_pool.tile([P, C], bf16, tag="woutT")
            btri = const_pool.tile([P, P], bf16, tag="btri")
            sel = const_pool.tile([65, P], bf16, tag="sel")
            ident = const_pool.tile([P, P], bf16, tag="ident")
            dummy = const_pool.tile([P, 512], bf16, tag="dummy")
            # weights + small consts go on the vector queue: a separate
            # DMA chain that runs parallel to the x chunks on sync, so the
            # V weights land at ~2us instead of behind 3 x-chunk transfers
            nc.scalar.dma_start(wqkvT[:, :, 128:256], wqkvT_d[:, :, 128:256])
            nc.scalar.dma_start(wqkvT[:, :, 0:128], wqkvT_d[:, :, 0:128])
            nc.scalar.dma_start(wqkvT[:, :, 256:384], wqkvT_d[:, :, 256:384])
            nc.scalar.dma_start(ident, ident_d[:])
            nc.scalar.dma_start(btri, btri_d[:])
            nc.scalar.dma_start(sel, sel_d[:])
            nc.scalar.dma_start(woutT, woutT_d[:])

            # ---- x, token-chunked (batch 1 chunks deferred into the
            # attention filler stream to keep early HBM read BW for batch 0)
            xb = []
            for b in range(B):
                xt = xb_pool.tile([P, NCHUNK, 8, CH], bf16, tag="xb",
                                  name=f"xb{b}")
                xb.append(xt)
            nc.sync.dma_start(xb[0][:, 0, :, 0:256], xT_d[:, 0, 0, :, 0:256])
            nc.sync.dma_start(xb[0][:, 0, :, 256:512],
                              xT_d[:, 0, 0, :, 256:512])
            for n in range(1, NCHUNK):
                nc.sync.dma_start(xb[0][:, n], xT_d[:, 0, n])

            # ---- PE warm-up (HAM: reach K=8/8 before real matmuls) ----
            nc.vector.memset(dummy, 0.0)
            wm = ps_pool.tile([P, 512], f32, tag="ps", name="wm")
            for _ in range(10):
                nc.tensor.matmul(wm, dummy[:, 0:128], dummy[:, 0:512],
                                 start=True, stop=True)

            # ---- per-batch working tiles ----
            qT, kT, vT, vh, attnT, rl2, l2, rl2b = ([], [], [], [], [], [],
                                                    [], [])
            for b in range(B):
                qT.append(qkv_pool.tile([P, T], bf16, tag="qT", name=f"qT{b}"))
                kT.append(qkv_pool.tile([P, T], bf16, tag="kT", name=f"kT{b}"))
                vT.append(qkv_pool.tile([P, T], bf16, tag="vT", name=f"vT{b}"))
                vh.append(qkv_pool.tile([P, 16, 2, 65], bf16, tag="vh",
                                        name=f"vh{b}"))
                attnT.append(attnT_pool.tile([P, T], bf16, tag="attnT",
                                             name=f"attnT{b}"))
                rl2.append(rl_pool.tile([65, T], f32, tag="rl2",
                                        name=f"rl2{b}"))
                l2.append(rl_pool.tile([65, T], f32, tag="l2",
                                       name=f"l2{b}"))
                rl2b.append(rl_pool.tile([65, T], bf16, tag="rl2b",
                                         name=f"rl2b{b}"))
                # rows 1-63 are never written; keep them finite for recip
                nc.vector.memset(l2[b], 1.0)
                # ones column (col 64) -> PV row 64 = softmax denominator
                nc.vector.memset(vh[b][:, :, :, 64], 1.0)

            # ---------- emission units ----------
            def emit_qkv(b, n, fi, copy_fn):
                """One projection (fi: 0=q,1=k,2=v) for token chunk n."""
                dest = (qT, kT, vT)[fi][b]
                nsl = slice(n * CH, (n + 1) * CH)
                pss = ps_pool.tile([P, 512], f32, tag="ps", name="pss")
                for c in range(8):
                    nc.tensor.matmul(
                        pss, wqkvT[:, c, fi * 128:(fi + 1) * 128],
                        xb[b][:, n, c, :], start=(c == 0), stop=(c == 7))
                copy_fn(dest[:, nsl], pss)

            def emit_transposes(b, n):
                """PE-transpose V token chunk n (4 t-blocks) into vh."""
                tp = ps_pool.tile([P, 4, P], bf16, tag="ps", name="tp")
                for j in range(4):
                    tb = 4 * n + j
                    nc.tensor.transpose(
                        tp[:, j, :], vT[b][:, tb * 128:(tb + 1) * 128], ident)
                for h in range(2):
                    nc.vector.tensor_copy(
                        vh[b][:, 4 * n:4 * n + 4, h, 0:64],
                        tp[:, :, h * 64:(h + 1) * 64])

            def emit_oproj(b, tb):
                ps_a = ps_pool.tile([P, 512], f32, tag="ps", name="opa")
                ps_b = ps_pool.tile([P, 512], f32, tag="ps", name="opb")
                tsl = slice(tb * 128, (tb + 1) * 128)
                nc.tensor.matmul(ps_a, attnT[b][:, tsl], woutT[:, 0:512],
                                 start=True, stop=True)
                nc.tensor.matmul(ps_b, attnT[b][:, tsl], woutT[:, 512:1024],
                                 start=True, stop=True)
                osb = osb_pool.tile([P, C], bf16, tag="osb")
                # keep batch-1 attention's scalar queue pure-exp: route both
                # halves to vector there; the tail (tb>=12) has scalar free
                if b == 1 and tb >= 12:
                    # tail: drain each bank with both engines concurrently
                    nc.scalar.copy(osb[:, 0:256], ps_a[:, 0:256])
                    nc.vector.tensor_copy(osb[:, 256:512], ps_a[:, 256:512])
                    nc.scalar.copy(osb[:, 512:768], ps_b[:, 0:256])
                    nc.vector.tensor_copy(osb[:, 768:1024], ps_b[:, 256:512])
                elif b == 0:
                    nc.scalar.copy(osb[:, 0:512], ps_a)
                    nc.vector.tensor_copy(osb[:, 512:1024], ps_b)
                else:
                    nc.vector.tensor_copy(osb[:, 0:512], ps_a)
                    nc.vector.tensor_copy(osb[:, 512:1024], ps_b)
                nc.gpsimd.dma_start(
                    out_d[(b * T + tb * 128):(b * T + (tb + 1) * 128), :], osb)

            def emit_rbnorm(b, qc):
                qsl = slice(qc * 512, (qc + 1) * 512)
                rb = ps_pool.tile([P, 512], f32, tag="ps", name="rb")
                nc.tensor.matmul(rb, sel[:, :], rl2b[b][:, qsl],
                                 start=True, stop=True)
                nc.vector.tensor_mul(attnT[b][:, qsl], attnT[b][:, qsl], rb)

            # filler queue: popped one unit per site inside the attention loop
            F = deque()

            def pop_F():
                if F:
                    F.popleft()()

            # filler order: V-transposes and remaining QKV chunks first (they
            # have deadlines inside batch-0 attention), then batch 1's QKV,
            # then (appended later, at qc ends) rbnorm+oproj units.
            F.append(lambda: nc.sync.dma_start(xb[1][:, 0], xT_d[:, 1, 0]))
            F.append(lambda: emit_transposes(0, 0))
            for n in range(1, NCHUNK):
                F.append(lambda n=n: nc.sync.dma_start(xb[1][:, n],
                                                       xT_d[:, 1, n]))
                for fi in (1, 0, 2):
                    F.append(lambda b=0, n=n, fi=fi:
                             emit_qkv(b, n, fi, nc.vector.tensor_copy))
                F.append(lambda n=n: emit_transposes(0, n))
            for n in range(NCHUNK):
                for fi in (1, 0, 2):
                    F.append(lambda b=1, n=n, fi=fi:
                             emit_qkv(b, n, fi, nc.vector.tensor_copy))
                F.append(lambda n=n: emit_transposes(1, n))

            # ---------- lead-in: batch 0 token chunk 0, half-chunks so
            # the PE starts on the first 256 tokens as soon as they land
            def emit_qkv0(fi):
                dest = (qT, kT, vT)[fi][0]
                pss = ps_pool.tile([P, 512], f32, tag="ps", name="pss")
                for half in range(2):
                    hsl = slice(half * 256, (half + 1) * 256)
                    for c in range(8):
                        nc.tensor.matmul(
                            pss[:, hsl],
                            wqkvT[:, c, fi * 128:(fi + 1) * 128],
                            xb[0][:, 0, c, hsl],
                            start=(c == 0), stop=(c == 7),
                            skip_group_check=True)
                nc.scalar.copy(dest[:, 0:CH], pss)

            for fi in (1, 0, 2):
                emit_qkv0(fi)

            # ---------- attention ----------
            pending_drain = [None]

            def make_drain(b, qc):
                qsl = slice(qc * 512, (qc + 1) * 512)

                def drain(pv):
                    # reciprocal_approx_* requires partition base 0: bounce
                    # the two denominator rows into l2 first. In batch 0 the
                    # scalar queue has slack (tensor-bound phase): split the
                    # serial chain across scalar+vector so the vector queue
                    # does not head-block the qkv-filler evacuations.
                    if b == 0:
                        nc.scalar.copy(l2[b][0:1, qsl], pv[0][64:65, :])
                        nc.vector.tensor_copy(l2[b][64:65, qsl],
                                              pv[1][64:65, :])
                    else:
                        for h in range(2):
                            nc.vector.tensor_copy(
                                l2[b][64 * h:64 * h + 1, qsl],
                                pv[h][64:65, :])
                    nc.vector.reciprocal_approx_fast(rl2[b][:, qsl],
                                                     l2[b][:, qsl])
                    if b == 0:
                        nc.scalar.copy(rl2b[b][:, qsl], rl2[b][:, qsl])
                    else:
                        nc.vector.tensor_copy(rl2b[b][:, qsl],
                                              rl2[b][:, qsl])
                    if b == 0:
                        nc.scalar.copy(attnT[b][0:64, qsl], pv[0][0:64, :])
                    else:
                        nc.vector.tensor_copy(attnT[b][0:64, qsl],
                                              pv[0][0:64, :])
                    nc.vector.tensor_copy(attnT[b][64:128, qsl],
                                          pv[1][0:64, :])
                    for tb in range(4 * qc + 3, 4 * qc - 1, -1):
                        F.appendleft(lambda b=b, tb=tb: emit_oproj(b, tb))
                    F.appendleft(lambda: emit_rbnorm(b, qc))
                return drain

            def attention(b):
                for qc in range(4):
                    nk = 4 * qc + 4
                    qsl = slice(qc * 512, (qc + 1) * 512)
                    pv = [pv_pool.tile([P, 512], f32, tag=f"pv{h}",
                                       name=f"pv{h}")
                          for h in range(2)]
                    prev = None
                    for g0 in range(0, nk, 2):
                        kbs = [g0, g0 + 1]
                        st = [st_pool.tile([P, 2, 512], f32, tag="st",
                                           name=f"st{h}")
                              for h in range(2)]
                        pt = [pt_pool.tile([P, 2, 512], bf16, tag="pt",
                                           name=f"pt{h}")
                              for h in range(2)]
                        # scores, heads interleaved for PE row-packing;
                        # diagonal blocks get the causal mask accumulated in
                        # as a -1e4 upper-triangle bias matmul (exp -> 0)
                        for j, kb in enumerate(kbs):
                            diag = kb >= 4 * qc
                            off = (kb - 4 * qc) * 128
                            for h in range(2):
                                hs = h * 64
                                nc.tensor.matmul(
                                    st[h][:, j, :],
                                    kT[b][hs:hs + 64,
                                          kb * 128:(kb + 1) * 128],
                                    qT[b][hs:hs + 64, qsl],
                                    start=True, stop=not diag)
                            if diag:
                                for h in range(2):
                                    nc.tensor.matmul(
                                        st[h][:, j, off:off + 128],
                                        ident, btri,
                                        start=False, stop=True,
                                        skip_group_check=True)
                        for h in range(2):
                            nc.scalar.activation(pt[h], st[h], Exp,
                                                 scale=SCALE)
                        if g0 == 0 and pending_drain[0] is not None:
                            pending_drain[0]()
                            pending_drain[0] = None
                        pop_F()
                        if prev is not None:
                            emit_pv(b, qc, nk, pv, *prev)
                        pop_F()
                        prev = (pt, kbs)
                    emit_pv(b, qc, nk, pv, *prev)
                    dr = make_drain(b, qc)
                    pending_drain[0] = lambda dr=dr, pv=pv: dr(pv)
                    pop_F()
                    pop_F()

            def emit_pv(b, qc, nk, pv, pt, kbs):
                for h in range(2):
                    for j, kb in enumerate(kbs):
                        off = max(0, (kb - 4 * qc) * 128)
                        nc.tensor.matmul(
                            pv[h][:65, off:512],
                            vh[b][:, kb, h, :],
                            pt[h][:, j, off:512],
                            start=(kb == 0), stop=(kb == nk - 1),
                            skip_group_check=True)

            attention(0)
            attention(1)
            if pending_drain[0] is not None:
                pending_drain[0]()
                pending_drain[0] = None
            while F:
                pop_F()

    nc.compile()
    return nc


def _get_compiled():
    global _COMPILED
    if _COMPILED is None:
        _COMPILED = _build()
    return _COMPILED


def make_core_inputs(x, w_qkv, w_out):
    """Host-side shard prep: returns list of per-core input dicts."""
    xf = np.asarray(x, dtype=np.float32).reshape(BT, C)
    # xT[ci, b, n, co, t] = x[b, n*CH + t, co*128 + ci]
    xT = np.ascontiguousarray(
        xf.T.reshape(8, P, B, NCHUNK, CH).transpose(1, 2, 3, 0, 4)
    ).astype(_bf16)

    btri = np.zeros((P, P), dtype=np.float32)
    kk, qq = np.meshgrid(np.arange(P), np.arange(P), indexing="ij")
    btri[kk > qq] = -1e4
    btri = btri.astype(_bf16)

    sel = np.zeros((65, P), dtype=_bf16)
    sel[0, 0:64] = 1.0
    sel[64, 64:128] = 1.0

    ident = np.eye(P, dtype=_bf16)

    w_qkv = np.asarray(w_qkv, dtype=np.float32)
    w_out = np.asarray(w_out, dtype=np.float32)

    ins = []
    for core in range(NCORES):
        r0 = 2 * core * D
        wsel = np.concatenate([
            w_qkv[r0:r0 + 128],
            w_qkv[C + r0:C + r0 + 128],
            w_qkv[2 * C + r0:2 * C + r0 + 128],
        ], axis=0)  # [384, 1024]
        wqkvT = np.ascontiguousarray(
            wsel.T.reshape(8, P, 384).transpose(1, 0, 2)).astype(_bf16)
        woutT = np.ascontiguousarray(
            w_out[:, core * P:(core + 1) * P].T).astype(_bf16)
        ins.append({
            "xT": xT,
            "wqkvT": wqkvT,
            "woutT": woutT,
            "btri": btri,
            "sel": sel,
            "ident": ident,
        })
    return ins


def kernel(x, w_qkv, w_out):
    global LAST_RESULTS
    from concourse.bass_utils import run_bass_kernel_spmd

    nc = _get_compiled()
    ins = make_core_inputs(x, w_qkv, w_out)
    trace = bool(os.environ.get("KERNEL_TRACE"))
    res = run_bass_kernel_spmd(nc, ins, core_ids=list(range(NCORES)),
                               trace=trace)
    LAST_RESULTS = res
    out = np.zeros((BT, C), dtype=np.float32)
    for r in res.results:
        out += r["out"]
    return out.reshape(B, T, C)
